# revision 28
# baseline (speedup 1.0000x reference)
"""DLRM embedding-lookup kernel for 8 TRN2 NeuronCores.

Strategy: data-parallel over the batch (B=16384 -> 2048 rows/core), with the
26 embedding tables ([26, 1M, 2] f32, 208MB) replicated into each core's HBM.
Each core does one table-major indirect-DMA gather (53,248 rows of 8B) plus
the tiny bottom/top MLPs entirely in feature-on-partition layout, so no
on-device transposes are needed:

  - host prep: idxt[t, b] = t*V + x_cat[b, t]  (int32, [26, 2048] per core)
               xdt = x_dense_shard.T           ([13, 2048])
               all MLP weights/biases packed into one [26, 25] tensor;
               top_w1 pre-split into d-rows / e-even-rows / e-odd-rows so the
               interleaved gather output can feed matmul directly.
  - gather: g[t, 2b:2b+2] = emb_flat[idxt[t,b]] via gpsimd indirect DMA,
    chunked along the batch so the top MLP pipelines behind the gather.
  - bottom MLP: [13,2048] -> [3,2048] -> [2,2048] on TensorE (f32r, full
    rate), bias+relu fused as one DVE tensor_scalar(add, max).
  - top MLP: h1 = w1d.T@d + w1e0.T@g_even + w1e1.T@g_odd (PSUM accumulation),
    then 4->2->1 with bias+relu / bias+sigmoid on ScalarE, batch chunked
    [512,512,512,256,256] (small tail chunk shortens the post-gather chain).
  - per-engine instruction order is pinned with ordering-only deps so the
    in-order engines process chunks in gather-arrival order (no head-of-line
    blocking).
"""

import numpy as np

import concourse.bacc as bacc
import concourse.bass as bass
import concourse.mybir as mybir
import concourse.tile as tile
from concourse.bass_utils import run_bass_kernel_spmd
from concourse.tile_rust import add_dep_helper

N_CORES = 8
B_FULL = 16384
N_DENSE = 13
T = 26
V = 1_000_000
E = 2

F32 = mybir.dt.float32
# float32r: same 32-bit storage as f32, but full-rate on TensorE (fp32 proper
# runs at 1/4 rate). The walrus BIR verifier requires every tensor feeding an
# f32r matmul to be f32r-typed, so the whole matmul-feeding chain uses F32R.
F32R = mybir.dt.float32r
I32 = mybir.dt.int32

RELU = mybir.ActivationFunctionType.Relu
SIGMOID = mybir.ActivationFunctionType.Sigmoid

# Column layout of the packed weight tensor wpack [T, WCOLS].
# Each entry: name -> (n_partitions, col_start, n_cols)
WPACK = {
    "bw1": (N_DENSE, 0, 3),
    "bb1": (3, 3, 1),
    "bw2": (3, 4, 2),
    "bb2": (2, 6, 1),
    "w1d": (2, 7, 4),
    "w1e0": (T, 11, 4),
    "w1e1": (T, 15, 4),
    "tb1": (4, 19, 1),
    "tw2": (4, 20, 2),
    "tb2": (2, 22, 1),
    "tw3": (2, 23, 1),
    "tb3": (1, 24, 1),
}
WCOLS = 25


def build_module(bs, v=V, mm_chunk=512, gather_splits_per_chunk=1, repeat=1,
                 chunks=None, single_out_dma=False):
    """Build the per-core Bass module for a batch shard of `bs` rows.

    repeat>1 re-emits the whole compute body N times inside one NEFF —
    used only for steady-state HW timing (marginal per-iteration cost).
    """
    nc = bacc.Bacc(trn_type="TRN2")

    emb = nc.declare_dram_parameter("emb", [T * v, E], F32R, isOutput=False)
    idxt = nc.declare_dram_parameter("idxt", [T, bs], I32, isOutput=False)
    xdt = nc.declare_dram_parameter("xdt", [N_DENSE, bs], F32R, isOutput=False)
    wpack = nc.declare_dram_parameter("wpack", [T, WCOLS], F32R, isOutput=False)
    out = nc.declare_dram_parameter("out", [1, bs], F32, isOutput=True)

    if chunks is None:
        chunks = [mm_chunk] * (bs // mm_chunk)
    assert sum(chunks) == bs
    spans = []
    off = 0
    for sz in chunks:
        spans.append((off, sz))
        off += sz
    nch = len(spans)

    with tile.TileContext(nc) as tc:
        with (
            tc.tile_pool(name="w", bufs=1) as wp,
            tc.tile_pool(name="data", bufs=1) as dp,
            tc.tile_pool(name="acts", bufs=2) as ap_,
            tc.tile_pool(name="psum", bufs=2, space="PSUM") as pp,
        ):
            # indices first: the gathers (the long pole) depend only on them.
            # split per chunk so the first gather starts after 1/nch of the DMA
            idx_s = dp.tile([T, bs], I32, tag="idx")
            o0, sz0 = spans[0]
            nc.sync.dma_start(out=idx_s[:, :sz0], in_=idxt[:, :sz0])
            if bs > sz0:
                nc.sync.dma_start(out=idx_s[:, sz0:], in_=idxt[:, sz0:])

            wp_s = wp.tile([T, WCOLS], F32R, tag="wpack")
            nc.sync.dma_start(out=wp_s[:], in_=wpack[:])

            def w(name):
                p, c0, ncol = WPACK[name]
                ap = wp_s[:p, c0 : c0 + ncol]
                # biases feed DVE/ACT as plain f32; weights stay f32r for PE
                if name in ("bb1", "bb2", "tb1", "tb2", "tb3"):
                    ap = ap.bitcast(F32)
                return ap

            xdt_s = dp.tile([N_DENSE, bs], F32R, tag="xdt")
            nc.sync.dma_start(out=xdt_s[:], in_=xdt[:])

            out_s = dp.tile([1, bs], F32, tag="outs")

            for _rep in range(repeat):
                emit_body(
                    nc, dp, pp, ap_, bs, spans, gather_splits_per_chunk,
                    emb, xdt_s, idx_s, out_s, out, w, single_out_dma,
                )

    nc.finalize()
    return nc


def emit_body(nc, dp, pp, ap_, bs, spans, gsp, emb, xdt_s, idx_s, out_s, out, w,
              single_out_dma=False):
    nch = len(spans)
    # In-order engines + data arriving in chunk order (the gathers drain the
    # single SWDGE queue FIFO) mean the only stall-free schedule is exactly
    # program order per engine. Chain each engine's instructions with
    # ordering-only deps so the Tile scheduler cannot reorder them.
    last_on = {}

    CHAIN_ENGINES = {mybir.EngineType.Activation, mybir.EngineType.PE}

    def chain(bi):
        eng = bi.ins.engine
        if eng not in CHAIN_ENGINES:
            return bi
        prev = last_on.get(eng)
        if prev is not None:
            add_dep_helper(bi.ins, prev, sync=False, reason="pin engine order")
        last_on[eng] = bi.ins
        return bi

    # Gathers first in program order: they are the long pole and depend only
    # on idx_s, so the Pool engine starts them immediately.
    g_tiles = []
    for c, (o, sz) in enumerate(spans):
        g = dp.tile([T, sz * E], F32R, tag=f"g{c}")
        g_tiles.append(g)
        for s in range(gsp):
            wdt = sz // gsp
            chain(nc.gpsimd.indirect_dma_start(
                out=g[:, s * wdt * E : (s + 1) * wdt * E],
                out_offset=None,
                in_=emb[:],
                in_offset=bass.IndirectOffsetOnAxis(
                    ap=idx_s[:, o + s * wdt : o + (s + 1) * wdt],
                    axis=0,
                ),
            ))

    # Bottom MLP over the full shard: xdT [13,bs] -> b1 [3,bs] -> dT [2,bs].
    # bias+relu fused as one DVE tensor_scalar (add, max 0) to keep ScalarE
    # free for the top MLP.
    b1 = dp.tile([3, bs], F32R, tag="b1")
    dT = dp.tile([2, bs], F32R, tag="dT")
    for c, (o, sz) in enumerate(spans):
        sl = slice(o, o + sz)
        p1 = pp.tile([3, sz], F32, tag="ps_bot")
        chain(nc.tensor.matmul(
            out=p1[:], lhsT=w("bw1"), rhs=xdt_s[:, sl], start=True, stop=True
        ))
        chain(nc.vector.tensor_scalar(
            out=b1[:, sl], in0=p1[:], scalar1=w("bb1"), scalar2=0.0,
            op0=mybir.AluOpType.add, op1=mybir.AluOpType.max,
        ))
        p2 = pp.tile([2, sz], F32, tag="ps_bot")
        chain(nc.tensor.matmul(
            out=p2[:], lhsT=w("bw2"), rhs=b1[:, sl], start=True, stop=True
        ))
        chain(nc.vector.tensor_scalar(
            out=dT[:, sl], in0=p2[:], scalar1=w("bb2"), scalar2=0.0,
            op0=mybir.AluOpType.add, op1=mybir.AluOpType.max,
        ))

    # Top MLP, software-pipelined: chunk c+1's layer-1 matmuls are emitted
    # (and pinned on PE) BEFORE chunk c's layer-2/3 matmuls, so when the last
    # gather lands PE starts its ph1 immediately instead of idling behind the
    # previous chunk's dependent chain. ACT stays depth-first per chunk.
    def ph1_mms(c):
        o, sz = spans[c]
        g = g_tiles[c]
        ph1 = pp.tile([4, sz], F32, tag="ps_h1")
        chain(nc.tensor.matmul(
            out=ph1[:], lhsT=w("w1d"), rhs=dT[:, o:o + sz], start=True, stop=False
        ))
        chain(nc.tensor.matmul(
            out=ph1[:], lhsT=w("w1e0"), rhs=g[:, 0::E], start=False, stop=False
        ))
        chain(nc.tensor.matmul(
            out=ph1[:], lhsT=w("w1e1"), rhs=g[:, 1::E], start=False, stop=True
        ))
        return ph1

    ph1s = {0: ph1_mms(0)}
    for c, (o, sz) in enumerate(spans):
        sl = slice(o, o + sz)
        if c not in ph1s:
            ph1s[c] = ph1_mms(c)

        h1s = ap_.tile([4, sz], F32R, tag="h1s")
        chain(nc.scalar.activation(out=h1s[:], in_=ph1s[c][:], func=RELU, bias=w("tb1")))

        ph2 = pp.tile([2, sz], F32, tag="ps_h2")
        chain(nc.tensor.matmul(
            out=ph2[:], lhsT=w("tw2"), rhs=h1s[:], start=True, stop=True
        ))
        h2s = ap_.tile([2, sz], F32R, tag="h2s")
        chain(nc.scalar.activation(out=h2s[:], in_=ph2[:], func=RELU, bias=w("tb2")))

        ph3 = pp.tile([1, sz], F32, tag="ps_h3")
        chain(nc.tensor.matmul(
            out=ph3[:], lhsT=w("tw3"), rhs=h2s[:], start=True, stop=True
        ))
        chain(nc.scalar.activation(
            out=out_s[:, sl], in_=ph3[:], func=SIGMOID, bias=w("tb3")
        ))
        if not single_out_dma:
            nc.scalar.dma_start(out=out[:, sl], in_=out_s[:, sl])
    if single_out_dma:
        nc.scalar.dma_start(out=out[:], in_=out_s[:])


def make_in_maps(inputs, bs, v=V, n_cores=N_CORES):
    """Host-side shard + preprocess. Returns list of per-core input dicts."""
    x_dense = np.asarray(inputs["x_dense"], dtype=np.float32)
    x_cat = np.asarray(inputs["x_cat"])
    emb = np.ascontiguousarray(np.asarray(inputs["emb"], dtype=np.float32)).reshape(
        T * v, E
    )

    top_w1 = np.asarray(inputs["top_w1"], dtype=np.float32)  # [54, 4]
    w1e = top_w1[2:].reshape(T, E, 4)

    pieces = {
        "bw1": np.asarray(inputs["bot_w1"], dtype=np.float32),
        "bb1": np.asarray(inputs["bot_b1"], dtype=np.float32).reshape(3, 1),
        "bw2": np.asarray(inputs["bot_w2"], dtype=np.float32),
        "bb2": np.asarray(inputs["bot_b2"], dtype=np.float32).reshape(2, 1),
        "w1d": top_w1[:2],
        "w1e0": w1e[:, 0],
        "w1e1": w1e[:, 1],
        "tb1": np.asarray(inputs["top_b1"], dtype=np.float32).reshape(4, 1),
        "tw2": np.asarray(inputs["top_w2"], dtype=np.float32),
        "tb2": np.asarray(inputs["top_b2"], dtype=np.float32).reshape(2, 1),
        "tw3": np.asarray(inputs["top_w3"], dtype=np.float32),
        "tb3": np.asarray(inputs["top_b3"], dtype=np.float32).reshape(1, 1),
    }
    wpack = np.zeros((T, WCOLS), dtype=np.float32)
    for name, (p, c0, ncol) in WPACK.items():
        arr = np.asarray(pieces[name], dtype=np.float32)
        assert arr.shape == (p, ncol), (name, arr.shape, (p, ncol))
        wpack[:p, c0 : c0 + ncol] = arr

    table_off = (np.arange(T, dtype=np.int64) * v)[:, None]  # [T, 1]
    in_maps = []
    for i in range(n_cores):
        s = slice(i * bs, (i + 1) * bs)
        idxt = (x_cat[s].astype(np.int64).T + table_off).astype(np.int32)
        in_maps.append(
            {
                "emb": emb,
                "wpack": wpack,
                "idxt": np.ascontiguousarray(idxt),
                "xdt": np.ascontiguousarray(x_dense[s].T),
            }
        )
    return in_maps


_NC_CACHE = {}


def _get_module(bs):
    if bs not in _NC_CACHE:
        _NC_CACHE[bs] = build_module(
            bs, chunks=[512, 512, 512, 256, 256], single_out_dma=True
        )
    return _NC_CACHE[bs]


def run(inputs, **spmd_kwargs):
    """Run the SPMD kernel; returns (full_output, BassKernelResults)."""
    bs = B_FULL // N_CORES
    nc = _get_module(bs)
    in_maps = make_in_maps(inputs, bs)
    res = run_bass_kernel_spmd(nc, in_maps, list(range(N_CORES)), **spmd_kwargs)
    out = np.concatenate([r["out"].reshape(bs) for r in res.results])
    return out.reshape(B_FULL, 1).astype(np.float32), res


def kernel(**inputs):
    return run(inputs)[0]
